# revision 54
# baseline (speedup 1.0000x reference)
"""Distributed Bass kernel for nn_Attention_12953621365048 (8 TRN2 NeuronCores).

Sharding: 2 batch-groups x 4 head-groups (3 heads/core).
  core c: batch b = c//4, heads 3*(c%4) .. 3*(c%4)+2
Per core: qkv/kv matmuls (transposed [dim, tok] layout, y tokens first so PE
starts early), RMSNorm + RoPE, attention with no-max softmax (scores bounded:
q,k RMSNorm'd) processing two q-chunks in lockstep so scalar-exp latency hides
under matmuls, 8-way AllToAll per head (warmed by a tiny kernel-start A2A),
then a 12-block unmasked projection. Cross-batch A2A shards are sent as exact
zeros (o * mk * 1/den with per-core mk in {0,1}) so the receiver folds the two
batch halves with one add and uses shared unmasked Wproj. Softmax denominators
accumulate in PSUM via quad-summed ones-matmuls; 1/den runs on DVE
(Newton-Raphson) keeping the attention scalar stream pure-Exp (ACT table-set
switches cost ~2.7us). Host side only shards/gathers.
"""

from contextlib import ExitStack

import numpy as np
import ml_dtypes

import concourse.bass as bass
import concourse.mybir as mybir
import concourse.tile as tile
from concourse import bacc
from concourse.bass_utils import run_bass_kernel_spmd

B, N, M, C, H = 2, 2048, 512, 1536, 12
HD = C // H           # 128 head dim
RD = 64               # partial rotary dim
EPS = 1e-6
NHL = 3               # heads per core
S = N + M             # 2560 kv tokens
KT = S // 128         # 20 kv tiles
NQC = N // 512        # 4 q-chunks of 512 (== A2A block count per batch)
CH = 1024             # qkv-phase token chunk
F32 = mybir.dt.float32
AF = mybir.ActivationFunctionType
ALU = mybir.AluOpType
BF16 = mybir.dt.bfloat16
NCT = C // 128        # 12 contraction tiles


def build_nc():
    nc = bacc.Bacc("TRN2", target_bir_lowering=False, debug=False, num_devices=8)

    xT = nc.dram_tensor("xT", [C, N], BF16, kind="ExternalInput").ap()
    yT = nc.dram_tensor("yT", [C, M], BF16, kind="ExternalInput").ap()
    wqkv = nc.dram_tensor("wqkv", [C, 3 * NHL * HD], BF16, kind="ExternalInput").ap()
    wkv = nc.dram_tensor("wkv", [C, 2 * NHL * HD], BF16, kind="ExternalInput").ap()
    wproj = nc.dram_tensor("wproj", [C, C], BF16, kind="ExternalInput").ap()
    wq = nc.dram_tensor("wq", [1, HD], F32, kind="ExternalInput").ap()
    wk = nc.dram_tensor("wk", [1, HD], F32, kind="ExternalInput").ap()
    cs = nc.dram_tensor("cs", [RD, N], BF16, kind="ExternalInput").ap()
    sn = nc.dram_tensor("sn", [RD, N], BF16, kind="ExternalInput").ap()
    ywT = nc.dram_tensor("ywT", [128, M // 128], F32, kind="ExternalInput").ap()
    bpr = nc.dram_tensor("bpr", [1, C], F32, kind="ExternalInput").ap()
    mk = nc.dram_tensor("mk", [128, 2], F32, kind="ExternalInput").ap()
    out = nc.dram_tensor("out", [512, C], BF16, kind="ExternalOutput").ap()

    with tile.TileContext(nc) as tc, ExitStack() as ctx:
        # ---------- outer (whole-kernel) pools ----------
        pers = ctx.enter_context(tc.tile_pool(name="persist", bufs=1))
        dram = ctx.enter_context(tc.tile_pool(name="dram", bufs=1, space="DRAM"))

        # collective warmup: absorb core launch skew + ring setup before the
        # first real A2A; contiguous 512B shards so the DMA queue isn't
        # clogged with tiny packets
        warm_in = dram.tile([8, 256], BF16)
        warm_out = dram.tile([8, 256], BF16)
        zwarm = pers.tile([1, 256], BF16, tag="zwarm")
        nc.vector.memset(zwarm[:], 0.0)
        for i in range(8):
            nc.sync.dma_start(warm_in[i : i + 1], zwarm[:])
        nc.gpsimd.collective_compute(
            "AllToAll",
            ALU.bypass,
            replica_groups=[[0, 1, 2, 3, 4, 5, 6, 7]],
            ins=[warm_in.opt()],
            outs=[warm_out.opt()],
        )

        onesb_sb = pers.tile([128, 1], BF16, tag="onesb")
        nc.vector.memset(onesb_sb[:], 1.0)
        eps_sb = pers.tile([1, 1], F32, tag="eps")
        nc.vector.memset(eps_sb[:], EPS)
        wq_sb = pers.tile([128, 1], F32, tag="wq")
        nc.sync.dma_start(wq_sb[:], wq.rearrange("o p -> p o"))
        wk_sb = pers.tile([128, 1], F32, tag="wk")
        nc.sync.dma_start(wk_sb[:], wk.rearrange("o p -> p o"))
        mk_sb = pers.tile([128, 2], F32, tag="mk")
        nc.sync.dma_start(mk_sb[:], mk)

        # attention bias per kv tile column: 0 for x tokens, log(clip(w)) for
        # y; the Ln itself is emitted right before the first attention Exp
        # (phase C top) so walrus can serve both from one ACT table set
        bias_sb = pers.tile([128, KT], F32, tag="bias")
        nc.vector.memset(bias_sb[:, 0 : N // 128], 0.0)
        ywT_sb = pers.tile([128, M // 128], F32, tag="ywT")
        nc.sync.dma_start(ywT_sb[:], ywT)
        ywc = pers.tile([128, M // 128], F32, tag="ywc")
        nc.vector.tensor_scalar_max(ywc[:], ywT_sb[:], 1e-4)

        # persistent activations
        qn = [pers.tile([128, N], BF16, tag=f"qn{t}", name=f"qn{t}") for t in range(NHL)]
        kn = [pers.tile([128, S], BF16, tag=f"kn{t}", name=f"kn{t}") for t in range(NHL)]
        v_sb = pers.tile([128, KT * NHL * HD], BF16, tag="v")  # [kv_tile, head, hd]

        outp = ctx.enter_context(tc.tile_pool(name="osb", bufs=2))

        # ---------- phase A/B: qkv + kv, norm, rope ----------
        with ExitStack() as ab:
            csn = ab.enter_context(tc.tile_pool(name="csn", bufs=1))
            wbig = ab.enter_context(tc.tile_pool(name="wbig", bufs=2))
            xtp = ab.enter_context(tc.tile_pool(name="xt", bufs=2))
            sqp = ab.enter_context(tc.tile_pool(name="sq", bufs=2))
            smallp = ab.enter_context(tc.tile_pool(name="small", bufs=3))
            brp = ab.enter_context(tc.tile_pool(name="bcast", bufs=2))
            ropep = ab.enter_context(tc.tile_pool(name="rope", bufs=1))
            psA = ab.enter_context(tc.tile_pool(name="psA", bufs=2, space="PSUM"))
            psV = ab.enter_context(tc.tile_pool(name="psV", bufs=2, space="PSUM"))
            psS = ab.enter_context(tc.tile_pool(name="psS", bufs=1, space="PSUM"))

            def norm_head(raw_ps, dst, w_sb, rope_q0, CHc):
                """RMSNorm over partition dim (HD) + optional RoPE; [128,CHc].

                Raw PSUM is evacuated to SBUF first so the square can run on
                DVE (two same-tile PSUM reads are illegal there) — the scalar
                engine then runs rsqrt-only in phase A, avoiding ~2.7us ACT
                table-set switches per norm, and the PSUM bank frees early.
                """
                rawsb = sqp.tile([128, CH], BF16, tag="raw", name="raw")[:, :CHc]
                nc.vector.tensor_copy(rawsb, raw_ps[:])
                sq = sqp.tile([128, CH], BF16, tag="sq", name="sq")[:, :CHc]
                nc.vector.tensor_mul(sq, rawsb, rawsb)
                ssq = psS.tile([1, CH], F32, tag="ssq", name="ssq")[:, :CHc]
                for h0 in range(0, CHc, 512):
                    hw = min(512, CHc - h0)
                    nc.tensor.matmul(
                        ssq[:, h0 : h0 + hw],
                        onesb_sb[:],
                        sq[:, h0 : h0 + hw],
                        start=True,
                        stop=True,
                    )
                inv = smallp.tile([1, CH], F32, tag="inv", name="inv")[:, :CHc]
                nc.scalar.activation(
                    inv, ssq, AF.Abs_reciprocal_sqrt, bias=eps_sb[:],
                    scale=1.0 / HD,
                )
                binv = brp.tile([128, CH], F32, tag="binv", name="binv")[:, :CHc]
                nc.gpsimd.partition_broadcast(binv, inv)
                nc.vector.scalar_tensor_tensor(
                    dst, rawsb, w_sb[:], binv, op0=ALU.mult, op1=ALU.mult
                )
                if rope_q0 is not None:
                    hf = RD // 2
                    csc = cs_sb[:, rope_q0 : rope_q0 + CHc]
                    snc = sn_sb[:, rope_q0 : rope_q0 + CHc]
                    sw = ropep.tile([RD, CH], BF16, tag="sw", name="sw")[:, :CHc]
                    nc.scalar.copy(sw[0:hf, :], dst[hf:RD, :])
                    nc.scalar.copy(sw[hf:RD, :], dst[0:hf, :])
                    ma = ropep.tile([RD, CH], BF16, tag="ma", name="ma")[:, :CHc]
                    mb = ropep.tile([RD, CH], BF16, tag="mb", name="mb")[:, :CHc]
                    nc.vector.tensor_mul(ma, dst[0:RD, :], csc)
                    nc.vector.tensor_mul(mb, sw, snc)
                    nc.vector.tensor_add(dst[0:RD, :], ma, mb)

            def qkv_chunk(src_sb, w_sb, nqh, q0, kdst_off, vt0, rope, CHc):
                """One CHc-token chunk: q (nqh heads), k (NHL heads), v (NHL heads)."""
                for t in range(nqh):
                    ps = psA.tile([128, CH], F32, tag="qk", name="qk")[:, :CHc]
                    for ct in range(NCT):
                        for h0 in range(0, CHc, 512):
                            hw = min(512, CHc - h0)
                            nc.tensor.matmul(
                                ps[:, h0 : h0 + hw],
                                w_sb[:, ct, t * HD : (t + 1) * HD],
                                src_sb[:, ct, h0 : h0 + hw],
                                start=(ct == 0),
                                stop=(ct == NCT - 1),
                            )
                    norm_head(
                        ps, qn[t][:, q0 : q0 + CHc], wq_sb,
                        q0 if rope else None, CHc,
                    )
                koff = nqh * HD
                for t in range(NHL):
                    ps = psA.tile([128, CH], F32, tag="qk", name="qk")[:, :CHc]
                    for ct in range(NCT):
                        for h0 in range(0, CHc, 512):
                            hw = min(512, CHc - h0)
                            nc.tensor.matmul(
                                ps[:, h0 : h0 + hw],
                                w_sb[:, ct, koff + t * HD : koff + (t + 1) * HD],
                                src_sb[:, ct, h0 : h0 + hw],
                                start=(ct == 0),
                                stop=(ct == NCT - 1),
                            )
                    norm_head(
                        ps,
                        kn[t][:, kdst_off : kdst_off + CHc],
                        wk_sb,
                        q0 if rope else None,
                        CHc,
                    )
                voff = (nqh + NHL) * HD
                for ts in range(CHc // 128):
                    ps = psV.tile([128, NHL * HD], F32, tag="vps")
                    for ct in range(NCT):
                        nc.tensor.matmul(
                            ps[:],
                            src_sb[:, ct, ts * 128 : (ts + 1) * 128],
                            w_sb[:, ct, voff : voff + NHL * HD],
                            start=(ct == 0),
                            stop=(ct == NCT - 1),
                        )
                    kvt = vt0 + ts
                    nc.vector.tensor_copy(
                        v_sb[:, kvt * NHL * HD : (kvt + 1) * NHL * HD], ps[:]
                    )

            # y first: smallest prefetch (wkv+yT = 2.8MB) on the SCALAR
            # engine's DMA queue (idle at start) so PE starts early, while
            # wqkv/x stream on the sync queue in parallel; x split across
            # both queues
            wkv_sb = wbig.tile([128, NCT, 2 * NHL * HD], BF16, tag="wkv", bufs=1)
            yt_sb = xtp.tile([128, NCT, M], BF16, tag="yt", bufs=1)
            for ct in range(NCT):
                nc.scalar.dma_start(
                    wkv_sb[:, ct, :], wkv[ct * 128 : (ct + 1) * 128, :]
                )
                nc.scalar.dma_start(
                    yt_sb[:, ct, :], yT[ct * 128 : (ct + 1) * 128, :]
                )
            wqkv_sb = wbig.tile([128, NCT, 3 * NHL * HD], BF16, tag="wbig", bufs=1)
            xts = []
            for qc in range(N // CH):
                xts.append(xtp.tile([128, NCT, CH], BF16, tag="xt", name=f"xt{qc}"))
            for ct in range(NCT):
                nc.sync.dma_start(
                    wqkv_sb[:, ct, :], wqkv[ct * 128 : (ct + 1) * 128, :]
                )
                for qc in range(N // CH):
                    nc.sync.dma_start(
                        xts[qc][:, ct, :],
                        xT[ct * 128 : (ct + 1) * 128, qc * CH : (qc + 1) * CH],
                    )
            cs_sb = csn.tile([RD, N], BF16, tag="cs")
            nc.sync.dma_start(cs_sb[:], cs)
            sn_sb = csn.tile([RD, N], BF16, tag="sn")
            nc.sync.dma_start(sn_sb[:], sn)
            qkv_chunk(yt_sb, wkv_sb, 0, 0, N, N // 128, rope=False, CHc=M)
            for qc in range(N // CH):
                q0 = qc * CH
                qkv_chunk(xts[qc], wqkv_sb, NHL, q0, q0, q0 // 128, rope=True, CHc=CH)

        # ---------- phase C: attention + per-head A2A + projection ----------
        nc.scalar.activation(bias_sb[:, N // 128 : KT], ywc[:], AF.Ln)
        bpr_sb = pers.tile([1, C], F32, tag="bpr")
        nc.sync.dma_start(bpr_sb[:], bpr)
        bb_sb = pers.tile([128, C], F32, tag="bb")
        nc.gpsimd.partition_broadcast(bb_sb[:], bpr_sb[:])

        a2a_ins = [
            dram.tile([2 * NQC, 128, 512], BF16, name=f"a2ai{t}") for t in range(NHL)
        ]
        a2a_outs = [
            dram.tile([2 * NQC, 128, 512], BF16, name=f"a2ao{t}") for t in range(NHL)
        ]

        def a2a_head(t):
            nc.gpsimd.collective_compute(
                "AllToAll",
                ALU.bypass,
                replica_groups=[[0, 1, 2, 3, 4, 5, 6, 7]],
                ins=[a2a_ins[t].opt()],
                outs=[a2a_outs[t].opt()],
            )

        wpre = ctx.enter_context(tc.tile_pool(name="wpre", bufs=3))
        pjp = ctx.enter_context(tc.tile_pool(name="pjp", bufs=2))
        accp = ctx.enter_context(tc.tile_pool(name="accp", bufs=1))

        def prefetch_w(t, eng=None):
            # shared unmasked Wproj: row block (t*4+p) = Wproj rows of
            # head (3p+t); per head t keep [128, 4, C]
            wp = wpre.tile([128, 4, C], BF16, tag="wpre", name=f"wpre{t}")
            for p in range(4):
                r0 = (t * 4 + p) * 128
                (eng or nc.sync).dma_start(wp[:, p, :], wproj[r0 : r0 + 128, :])
            return wp

        with ExitStack() as pc:
            expp = pc.enter_context(tc.tile_pool(name="exp", bufs=6))
            exsp = pc.enter_context(tc.tile_pool(name="exs", bufs=4))
            brp2 = pc.enter_context(tc.tile_pool(name="bcast2", bufs=2))
            smallc = pc.enter_context(tc.tile_pool(name="smallc", bufs=2))
            avsp = pc.enter_context(tc.tile_pool(name="avs", bufs=2))
            psSc = pc.enter_context(tc.tile_pool(name="psSc", bufs=1, space="PSUM"))
            psAv = pc.enter_context(tc.tile_pool(name="psAv", bufs=1, space="PSUM"))
            psDen = pc.enter_context(tc.tile_pool(name="psDen", bufs=1, space="PSUM"))

            pend = {"cb": None}

            def flush_pend():
                cb = pend["cb"]
                if cb is not None:
                    pend["cb"] = None
                    cb()

            def attention_head(t, after_pair0=None):
                # two q-chunks (A/B) in lockstep: PE alternates between the
                # chunks so scalar-exp latency hides under matmuls, and each
                # kn/v stationary tile is reused for two matmuls
                for qp in range(2):
                    qA, qB = 2 * qp, 2 * qp + 1
                    avA = psAv.tile([128, 512], F32, tag="avA", name=f"avA{t}{qp}")
                    avB = psAv.tile([128, 512], F32, tag="avB", name=f"avB{t}{qp}")
                    den = psDen.tile([1, 1024], F32, tag="den", name=f"den{t}{qp}")
                    for kp in range(KT // 2):
                        scA = psSc.tile([128, 1024], F32, tag="scA")
                        scB = psSc.tile([128, 1024], F32, tag="scB")
                        for kh in range(2):
                            kt = 2 * kp + kh
                            for qc, sct in ((qA, scA), (qB, scB)):
                                nc.tensor.matmul(
                                    sct[:, kh * 512 : (kh + 1) * 512],
                                    kn[t][:, kt * 128 : (kt + 1) * 128],
                                    qn[t][:, qc * 512 : (qc + 1) * 512],
                                    start=True,
                                    stop=True,
                                )
                        if kp == 0:
                            # previous pair's deferred den finals + epilogue:
                            # emitted after this pair's first score matmuls so
                            # the PE never stalls on the vector-side quad sums
                            # (the new pair's first den write is at kp==1, so
                            # buffer-reuse ordering stays correct)
                            flush_pend()
                        if kp == 2 and qp == 0 and after_pair0 is not None:
                            after_pair0()
                        exA = expp.tile([128, 1024], BF16, tag="ex", name="exA")
                        exB = expp.tile([128, 1024], BF16, tag="ex", name="exB")
                        for ext, sct in ((exA, scA), (exB, scB)):
                            if kp < 8:
                                nc.scalar.activation(
                                    ext[:], sct[:], AF.Exp, bias=bias_sb[:, 0:1]
                                )
                            else:
                                for kh in range(2):
                                    kt = 2 * kp + kh
                                    nc.scalar.activation(
                                        ext[:, kh * 512 : (kh + 1) * 512],
                                        sct[:, kh * 512 : (kh + 1) * 512],
                                        AF.Exp,
                                        bias=bias_sb[:, kt : kt + 1],
                                    )
                        for kh in range(2):
                            kt = 2 * kp + kh
                            vsl = v_sb[
                                :,
                                kt * NHL * HD + t * HD : kt * NHL * HD + (t + 1) * HD,
                            ]
                            for avt, ext in ((avA, exA), (avB, exB)):
                                nc.tensor.matmul(
                                    avt[:],
                                    vsl,
                                    ext[:, kh * 512 : (kh + 1) * 512],
                                    start=(kt == 0),
                                    stop=(kt == KT - 1),
                                )
                        if kp == KT // 2 - 1:
                            # evacuate av PSUM first thing so the next pair's
                            # matmuls only wait on these copies
                            avsA = avsp.tile([128, 512], F32, tag="avsA")
                            nc.vector.tensor_copy(avsA[:], avA[:])
                            avsB = avsp.tile([128, 512], F32, tag="avsB")
                            nc.vector.tensor_copy(avsB[:], avB[:])
                        # den: pair then quad sums on vector, one accumulating
                        # den matmul per chunk per quad (PSUM-accumulated)
                        exsA = exsp.tile([128, 512], BF16, tag="exsA")
                        nc.vector.tensor_add(exsA[:], exA[:, 0:512], exA[:, 512:1024])
                        exsB = exsp.tile([128, 512], BF16, tag="exsB")
                        nc.vector.tensor_add(exsB[:], exB[:, 0:512], exB[:, 512:1024])
                        if kp % 2 == 0:
                            prevA, prevB = exsA, exsB
                        else:
                            exqA = exsp.tile([128, 512], BF16, tag="exqA", bufs=3)
                            nc.vector.tensor_add(exqA[:], prevA[:], exsA[:])
                            exqB = exsp.tile([128, 512], BF16, tag="exqB", bufs=3)
                            nc.vector.tensor_add(exqB[:], prevB[:], exsB[:])
                            if kp < KT // 2 - 1:
                                nc.tensor.matmul(
                                    den[:, 0:512], onesb_sb[:], exqA[:],
                                    start=(kp == 1), stop=False,
                                )
                                nc.tensor.matmul(
                                    den[:, 512:1024], onesb_sb[:], exqB[:],
                                    start=(kp == 1), stop=False,
                                )

                    def mk_ep(den=den, exqA=exqA, exqB=exqB, avsA=avsA,
                              avsB=avsB, t=t, qA=qA, qB=qB):
                        def ep():
                            nc.tensor.matmul(
                                den[:, 0:512], onesb_sb[:], exqA[:],
                                start=False, stop=True,
                            )
                            nc.tensor.matmul(
                                den[:, 512:1024], onesb_sb[:], exqB[:],
                                start=False, stop=True,
                            )
                            # 1/den via DVE Newton-Raphson approx (~2 ULP):
                            # keeps the scalar engine pure-Exp (an ACT
                            # table-set switch costs ~2.7us and exp/recip
                            # cannot share a table set)
                            rsc = smallc.tile([1, 1024], F32, tag="rsc")
                            invd = smallc.tile([1, 1024], F32, tag="invd")
                            nc.vector.reciprocal_approx_accurate(
                                invd[:], den[:], rsc[:]
                            )
                            bden = brp2.tile([128, 1024], F32, tag="bden")
                            nc.gpsimd.partition_broadcast(bden[:], invd[:])
                            # o{0,1} = av * mk{0,1} * bden, per-core mk in
                            # {0, 1}: exact-zero blocks go to the other
                            # batch's cores, letting the receiver fold batch
                            # halves with a plain add
                            for qc, avt, b0 in ((qA, avsA, 0), (qB, avsB, 512)):
                                o0 = outp.tile([128, 512], BF16, tag="o0")
                                nc.vector.scalar_tensor_tensor(
                                    o0[:], avt[:], mk_sb[:, 0:1],
                                    bden[:, b0 : b0 + 512],
                                    op0=ALU.mult, op1=ALU.mult,
                                )
                                o1 = outp.tile([128, 512], BF16, tag="o1")
                                nc.vector.scalar_tensor_tensor(
                                    o1[:], avt[:], mk_sb[:, 1:2],
                                    bden[:, b0 : b0 + 512],
                                    op0=ALU.mult, op1=ALU.mult,
                                )
                                nc.sync.dma_start(a2a_ins[t][qc], o0[:])
                                nc.sync.dma_start(a2a_ins[t][NQC + qc], o1[:])
                        return ep

                    pend["cb"] = mk_ep()

            def pj_fold(t, eng=None):
                # A2A result -> SBUF + fold batch halves (one half is zero);
                # emitted as early as the A2A completion allows so the tail
                # has only matmuls left
                pj_t = pjp.tile([128, 2 * NQC, 512], BF16, tag="pj", name=f"pj{t}")
                (eng or nc.sync).dma_start(
                    pj_t[:], a2a_outs[t].rearrange("i p q -> p i q")
                )
                pjs = pjp.tile([128, 4, 512], BF16, tag="pjs", name=f"pjs{t}")
                for p in range(4):
                    nc.vector.tensor_add(
                        pjs[:, p, :], pj_t[:, p, :], pj_t[:, 4 + p, :]
                    )
                return pjs

            acc = [
                accp.tile([128, 512], F32, tag=f"acc{i}", name=f"acc{i}")
                for i in range(12)
            ]

            def proj_partial(t, wp):
                # proj PSUM comes from the attention av rings (same shape/tag)
                # so there is no pool-scope transition barrier before the tail
                pjs = pjf[t]
                seq = 0
                for tcc in range(4):
                    for fc in range(3):
                        pp = psAv.tile(
                            [128, 512], F32,
                            tag=("avA" if seq % 2 == 0 else "avB"),
                            name=f"pp{t}_{fc}_{tcc}",
                        )
                        seq += 1
                        for p in range(4):
                            nc.tensor.matmul(
                                pp[:],
                                pjs[:, p, tcc * 128 : (tcc + 1) * 128],
                                wp[:, p, fc * 512 : (fc + 1) * 512],
                                start=(p == 0),
                                stop=(p == 3),
                            )
                        a = acc[fc * 4 + tcc]
                        if t == 0:
                            nc.vector.tensor_tensor(
                                a[:], pp[:],
                                bb_sb[:, fc * 512 : (fc + 1) * 512],
                                ALU.add,
                            )
                        elif t == 1:
                            nc.vector.tensor_add(a[:], a[:], pp[:])
                        else:
                            ob = outp.tile([128, 512], BF16, tag="ob")
                            nc.vector.tensor_add(ob[:], a[:], pp[:])
                            nc.sync.dma_start(
                                out[
                                    tcc * 128 : (tcc + 1) * 128,
                                    fc * 512 : (fc + 1) * 512,
                                ],
                                ob[:],
                            )

            wp0 = prefetch_w(0)
            attention_head(0)
            wp1 = prefetch_w(1)
            attention_head(1, after_pair0=lambda: a2a_head(0))
            # wp2/pj0 transfers go via the scalar engine's DMA queue so they
            # don't contend with o/a2a traffic on the sync queue at the
            # head1->head2 boundary
            wp2 = prefetch_w(2, eng=nc.scalar)
            pjf = {}

            def mid2():
                a2a_head(1)
                # a2a0 completed during head 1: fold it here so the tail's
                # first proj matmuls have their data the moment attention ends
                pjf[0] = pj_fold(0, eng=nc.scalar)

            attention_head(2, after_pair0=mid2)
            flush_pend()
            pjf[1] = pj_fold(1)
            a2a_head(2)
            proj_partial(0, wp0)
            proj_partial(1, wp1)
            # fold for head 2 emitted only now: its vector adds wait on the
            # last A2A and must not block proj0/1's accumulation adds
            pjf[2] = pj_fold(2)
            proj_partial(2, wp2)
    nc.compile()
    return nc


_NC_CACHE = {}


def _get_nc():
    if "nc" not in _NC_CACHE:
        _NC_CACHE["nc"] = build_nc()
    return _NC_CACHE["nc"]


def make_in_maps(x, y, pos, y_token_weights, Wqkv, Wkv, q_norm_w, k_norm_w, Wproj, bproj):
    f = np.float32
    c32 = pos[:, :, 0].T
    s32 = pos[:, :, 1].T
    csT = np.ascontiguousarray(
        np.concatenate([c32, c32], 0).astype(ml_dtypes.bfloat16))   # [64, N]
    snT = np.ascontiguousarray(
        np.concatenate([-s32, s32], 0).astype(ml_dtypes.bfloat16))  # [64, N]
    wqs = (np.asarray(q_norm_w, dtype=f) * np.float32(HD) ** -0.5).reshape(1, HD)
    wkk = np.asarray(k_norm_w, dtype=f).reshape(1, HD)
    Wp = np.asarray(Wproj, dtype=f)
    # shared unmasked proj weights: row block (t*4+p) = rows of head (3p+t)
    wproj_tg = np.concatenate(
        [Wp[(3 * p + t) * 128 : (3 * p + t + 1) * 128, :]
         for t in range(NHL) for p in range(4)],
        axis=0,
    ).astype(ml_dtypes.bfloat16)
    wproj_tg = np.ascontiguousarray(wproj_tg)
    in_maps = []
    for c in range(8):
        b, g = c // 4, c % 4
        heads = [3 * g + i for i in range(NHL)]
        qcols = [Wqkv[:, h * HD : (h + 1) * HD] for h in heads]
        kcols = [Wqkv[:, C + h * HD : C + (h + 1) * HD] for h in heads]
        vcols = [Wqkv[:, 2 * C + h * HD : 2 * C + (h + 1) * HD] for h in heads]
        wqkv_c = np.ascontiguousarray(
            np.concatenate(qcols + kcols + vcols, axis=1), dtype=f
        )
        kcols2 = [Wkv[:, h * HD : (h + 1) * HD] for h in heads]
        vcols2 = [Wkv[:, C + h * HD : C + (h + 1) * HD] for h in heads]
        wkv_c = np.ascontiguousarray(np.concatenate(kcols2 + vcols2, axis=1), dtype=f)
        mk_c = np.zeros((128, 2), dtype=f)
        mk_c[:, b] = 1.0
        in_maps.append(
            {
                "xT": np.ascontiguousarray(np.asarray(x)[b].T.astype(ml_dtypes.bfloat16)),
                "yT": np.ascontiguousarray(np.asarray(y)[b].T.astype(ml_dtypes.bfloat16)),
                "wqkv": wqkv_c.astype(ml_dtypes.bfloat16),
                "wkv": wkv_c.astype(ml_dtypes.bfloat16),
                "wproj": wproj_tg,
                "wq": np.ascontiguousarray(wqs),
                "wk": np.ascontiguousarray(wkk),
                "cs": csT,
                "sn": snT,
                "ywT": np.ascontiguousarray(
                    np.asarray(y_token_weights)[b].reshape(M // 128, 128).T, dtype=f
                ),
                "bpr": np.asarray(bproj, dtype=f).reshape(1, C),
                "mk": mk_c,
            }
        )
    return in_maps


def kernel(x, y, pos, y_token_weights, Wqkv, Wkv, q_norm_w, k_norm_w, Wproj, bproj,
           _trace=False):
    x = np.asarray(x, dtype=np.float32)
    y = np.asarray(y, dtype=np.float32)
    pos = np.asarray(pos, dtype=np.float32)
    y_token_weights = np.asarray(y_token_weights, dtype=np.float32)
    nc = _get_nc()
    in_maps = make_in_maps(
        x, y, pos, y_token_weights,
        np.asarray(Wqkv), np.asarray(Wkv), np.asarray(q_norm_w),
        np.asarray(k_norm_w), np.asarray(Wproj), np.asarray(bproj),
    )
    res = run_bass_kernel_spmd(nc, in_maps, core_ids=list(range(8)), trace=_trace)
    outp = np.zeros((B, N, C), dtype=np.float32)
    for c in range(8):
        b, g = c // 4, c % 4
        outp[b, g * 512 : (g + 1) * 512, :] = np.asarray(
            res.results[c]["out"]
        ).astype(np.float32)
    if _trace:
        return outp, res
    return outp


# revision 56
# speedup vs baseline: 1.2126x; 1.2126x over previous
"""Distributed Bass kernel for nn_Attention_12953621365048 (8 TRN2 NeuronCores).

Sharding: 2 batch-groups x 4 head-groups (3 heads/core).
  core c: batch b = c//4, heads 3*(c%4) .. 3*(c%4)+2
Per core: qkv/kv matmuls (transposed [dim, tok] layout, y tokens first so PE
starts early), RMSNorm + RoPE, attention with no-max softmax (scores bounded:
q,k RMSNorm'd) processing two q-chunks in lockstep so scalar-exp latency hides
under matmuls, 8-way AllToAll per head (warmed by a tiny kernel-start A2A),
then a 12-block unmasked projection. Cross-batch A2A shards are sent as exact
zeros (o * mk * 1/den with per-core mk in {0,1}) so the receiver folds the two
batch halves with one add and uses shared unmasked Wproj. Softmax denominators
accumulate in PSUM via quad-summed ones-matmuls; 1/den runs on DVE
(Newton-Raphson) keeping the attention scalar stream pure-Exp (ACT table-set
switches cost ~2.7us). Host side only shards/gathers.
"""

from contextlib import ExitStack

import numpy as np
import ml_dtypes

import concourse.bass as bass
import concourse.mybir as mybir
import concourse.tile as tile
from concourse import bacc
from concourse.bass_utils import run_bass_kernel_spmd

B, N, M, C, H = 2, 2048, 512, 1536, 12
HD = C // H           # 128 head dim
RD = 64               # partial rotary dim
EPS = 1e-6
NHL = 3               # heads per core
S = N + M             # 2560 kv tokens
KT = S // 128         # 20 kv tiles
NQC = N // 512        # 4 q-chunks of 512 (== A2A block count per batch)
CH = 1024             # qkv-phase token chunk
F32 = mybir.dt.float32
AF = mybir.ActivationFunctionType
ALU = mybir.AluOpType
BF16 = mybir.dt.bfloat16
NCT = C // 128        # 12 contraction tiles


def build_nc():
    nc = bacc.Bacc("TRN2", target_bir_lowering=False, debug=False, num_devices=8)

    xT = nc.dram_tensor("xT", [C, N], BF16, kind="ExternalInput").ap()
    yT = nc.dram_tensor("yT", [C, M], BF16, kind="ExternalInput").ap()
    wqkv = nc.dram_tensor("wqkv", [C, 3 * NHL * HD], BF16, kind="ExternalInput").ap()
    wkv = nc.dram_tensor("wkv", [C, 2 * NHL * HD], BF16, kind="ExternalInput").ap()
    wproj = nc.dram_tensor("wproj", [C, C], BF16, kind="ExternalInput").ap()
    wq = nc.dram_tensor("wq", [1, HD], F32, kind="ExternalInput").ap()
    wk = nc.dram_tensor("wk", [1, HD], F32, kind="ExternalInput").ap()
    cs = nc.dram_tensor("cs", [RD, N], BF16, kind="ExternalInput").ap()
    sn = nc.dram_tensor("sn", [RD, N], BF16, kind="ExternalInput").ap()
    ywT = nc.dram_tensor("ywT", [128, M // 128], F32, kind="ExternalInput").ap()
    bpr = nc.dram_tensor("bpr", [1, C], F32, kind="ExternalInput").ap()
    mk = nc.dram_tensor("mk", [128, 2], F32, kind="ExternalInput").ap()
    out = nc.dram_tensor("out", [512, C], BF16, kind="ExternalOutput").ap()

    with tile.TileContext(nc) as tc, ExitStack() as ctx:
        # ---------- outer (whole-kernel) pools ----------
        pers = ctx.enter_context(tc.tile_pool(name="persist", bufs=1))
        dram = ctx.enter_context(tc.tile_pool(name="dram", bufs=1, space="DRAM"))

        # collective warmup: absorb core launch skew + ring setup before the
        # first real A2A; contiguous 512B shards so the DMA queue isn't
        # clogged with tiny packets
        warm_in = dram.tile([8, 256], BF16)
        warm_out = dram.tile([8, 256], BF16)
        zwarm = pers.tile([1, 256], BF16, tag="zwarm")
        nc.vector.memset(zwarm[:], 0.0)
        for i in range(8):
            nc.sync.dma_start(warm_in[i : i + 1], zwarm[:])
        nc.gpsimd.collective_compute(
            "AllToAll",
            ALU.bypass,
            replica_groups=[[0, 1, 2, 3, 4, 5, 6, 7]],
            ins=[warm_in.opt()],
            outs=[warm_out.opt()],
        )

        onesb_sb = pers.tile([128, 1], BF16, tag="onesb")
        nc.vector.memset(onesb_sb[:], 1.0)
        eps_sb = pers.tile([1, 1], F32, tag="eps")
        nc.vector.memset(eps_sb[:], EPS)
        wq_sb = pers.tile([128, 1], F32, tag="wq")
        nc.sync.dma_start(wq_sb[:], wq.rearrange("o p -> p o"))
        wk_sb = pers.tile([128, 1], F32, tag="wk")
        nc.sync.dma_start(wk_sb[:], wk.rearrange("o p -> p o"))
        mk_sb = pers.tile([128, 2], F32, tag="mk")
        nc.sync.dma_start(mk_sb[:], mk)

        # attention bias per kv tile column: 0 for x tokens, log(clip(w)) for
        # y; the Ln itself is emitted right before the first attention Exp
        # (phase C top) so walrus can serve both from one ACT table set
        bias_sb = pers.tile([128, KT], F32, tag="bias")
        nc.vector.memset(bias_sb[:, 0 : N // 128], 0.0)
        ywT_sb = pers.tile([128, M // 128], F32, tag="ywT")
        nc.sync.dma_start(ywT_sb[:], ywT)
        ywc = pers.tile([128, M // 128], F32, tag="ywc")
        nc.vector.tensor_scalar_max(ywc[:], ywT_sb[:], 1e-4)

        # persistent activations
        qn = [pers.tile([128, N], BF16, tag=f"qn{t}", name=f"qn{t}") for t in range(NHL)]
        kn = [pers.tile([128, S], BF16, tag=f"kn{t}", name=f"kn{t}") for t in range(NHL)]
        v_sb = pers.tile([128, KT * NHL * HD], BF16, tag="v")  # [kv_tile, head, hd]

        outp = ctx.enter_context(tc.tile_pool(name="osb", bufs=2))

        # ---------- phase A/B: qkv + kv, norm, rope ----------
        with ExitStack() as ab:
            csn = ab.enter_context(tc.tile_pool(name="csn", bufs=1))
            wbig = ab.enter_context(tc.tile_pool(name="wbig", bufs=2))
            xtp = ab.enter_context(tc.tile_pool(name="xt", bufs=2))
            sqp = ab.enter_context(tc.tile_pool(name="sq", bufs=2))
            smallp = ab.enter_context(tc.tile_pool(name="small", bufs=3))
            brp = ab.enter_context(tc.tile_pool(name="bcast", bufs=2))
            ropep = ab.enter_context(tc.tile_pool(name="rope", bufs=1))
            psA = ab.enter_context(tc.tile_pool(name="psA", bufs=2, space="PSUM"))
            psV = ab.enter_context(tc.tile_pool(name="psV", bufs=2, space="PSUM"))
            psS = ab.enter_context(tc.tile_pool(name="psS", bufs=1, space="PSUM"))

            def norm_head(raw_ps, dst, w_sb, rope_q0, CHc):
                """RMSNorm over partition dim (HD) + optional RoPE; [128,CHc]."""
                sq = sqp.tile([128, CH], BF16, tag="sq", name="sq")[:, :CHc]
                nc.scalar.activation(sq, raw_ps[:], AF.Square)
                ssq = psS.tile([1, CH], F32, tag="ssq", name="ssq")[:, :CHc]
                for h0 in range(0, CHc, 512):
                    hw = min(512, CHc - h0)
                    nc.tensor.matmul(
                        ssq[:, h0 : h0 + hw],
                        onesb_sb[:],
                        sq[:, h0 : h0 + hw],
                        start=True,
                        stop=True,
                    )
                inv = smallp.tile([1, CH], F32, tag="inv", name="inv")[:, :CHc]
                nc.scalar.activation(
                    inv, ssq, AF.Abs_reciprocal_sqrt, bias=eps_sb[:],
                    scale=1.0 / HD,
                )
                binv = brp.tile([128, CH], F32, tag="binv", name="binv")[:, :CHc]
                nc.gpsimd.partition_broadcast(binv, inv)
                nc.vector.scalar_tensor_tensor(
                    dst, raw_ps[:], w_sb[:], binv, op0=ALU.mult, op1=ALU.mult
                )
                if rope_q0 is not None:
                    hf = RD // 2
                    csc = cs_sb[:, rope_q0 : rope_q0 + CHc]
                    snc = sn_sb[:, rope_q0 : rope_q0 + CHc]
                    sw = ropep.tile([RD, CH], BF16, tag="sw", name="sw")[:, :CHc]
                    nc.scalar.copy(sw[0:hf, :], dst[hf:RD, :])
                    nc.scalar.copy(sw[hf:RD, :], dst[0:hf, :])
                    ma = ropep.tile([RD, CH], BF16, tag="ma", name="ma")[:, :CHc]
                    mb = ropep.tile([RD, CH], BF16, tag="mb", name="mb")[:, :CHc]
                    nc.vector.tensor_mul(ma, dst[0:RD, :], csc)
                    nc.vector.tensor_mul(mb, sw, snc)
                    nc.vector.tensor_add(dst[0:RD, :], ma, mb)

            def qkv_chunk(src_sb, w_sb, nqh, q0, kdst_off, vt0, rope, CHc):
                """One CHc-token chunk: q (nqh heads), k (NHL heads), v (NHL heads)."""
                for t in range(nqh):
                    ps = psA.tile([128, CH], F32, tag="qk", name="qk")[:, :CHc]
                    for ct in range(NCT):
                        for h0 in range(0, CHc, 512):
                            hw = min(512, CHc - h0)
                            nc.tensor.matmul(
                                ps[:, h0 : h0 + hw],
                                w_sb[:, ct, t * HD : (t + 1) * HD],
                                src_sb[:, ct, h0 : h0 + hw],
                                start=(ct == 0),
                                stop=(ct == NCT - 1),
                            )
                    norm_head(
                        ps, qn[t][:, q0 : q0 + CHc], wq_sb,
                        q0 if rope else None, CHc,
                    )
                koff = nqh * HD
                for t in range(NHL):
                    ps = psA.tile([128, CH], F32, tag="qk", name="qk")[:, :CHc]
                    for ct in range(NCT):
                        for h0 in range(0, CHc, 512):
                            hw = min(512, CHc - h0)
                            nc.tensor.matmul(
                                ps[:, h0 : h0 + hw],
                                w_sb[:, ct, koff + t * HD : koff + (t + 1) * HD],
                                src_sb[:, ct, h0 : h0 + hw],
                                start=(ct == 0),
                                stop=(ct == NCT - 1),
                            )
                    norm_head(
                        ps,
                        kn[t][:, kdst_off : kdst_off + CHc],
                        wk_sb,
                        q0 if rope else None,
                        CHc,
                    )
                voff = (nqh + NHL) * HD
                for ts in range(CHc // 128):
                    ps = psV.tile([128, NHL * HD], F32, tag="vps")
                    for ct in range(NCT):
                        nc.tensor.matmul(
                            ps[:],
                            src_sb[:, ct, ts * 128 : (ts + 1) * 128],
                            w_sb[:, ct, voff : voff + NHL * HD],
                            start=(ct == 0),
                            stop=(ct == NCT - 1),
                        )
                    kvt = vt0 + ts
                    nc.vector.tensor_copy(
                        v_sb[:, kvt * NHL * HD : (kvt + 1) * NHL * HD], ps[:]
                    )

            # y first: smallest prefetch (wkv+yT = 2.8MB) on the SCALAR
            # engine's DMA queue (idle at start) so PE starts early, while
            # wqkv/x stream on the sync queue in parallel; x split across
            # both queues
            wkv_sb = wbig.tile([128, NCT, 2 * NHL * HD], BF16, tag="wkv", bufs=1)
            yt_sb = xtp.tile([128, NCT, M], BF16, tag="yt", bufs=1)
            for ct in range(NCT):
                nc.sync.dma_start(
                    wkv_sb[:, ct, :], wkv[ct * 128 : (ct + 1) * 128, :]
                )
                nc.scalar.dma_start(
                    yt_sb[:, ct, :], yT[ct * 128 : (ct + 1) * 128, :]
                )
            wqkv_sb = wbig.tile([128, NCT, 3 * NHL * HD], BF16, tag="wbig", bufs=1)
            xts = []
            for qc in range(N // CH):
                xts.append(xtp.tile([128, NCT, CH], BF16, tag="xt", name=f"xt{qc}"))
            for ct in range(NCT):
                weng = nc.sync if ct % 2 == 0 else nc.scalar
                weng.dma_start(
                    wqkv_sb[:, ct, :], wqkv[ct * 128 : (ct + 1) * 128, :]
                )
                for qc in range(N // CH):
                    xeng = nc.scalar if (ct + qc) % 2 == 0 else nc.sync
                    xeng.dma_start(
                        xts[qc][:, ct, :],
                        xT[ct * 128 : (ct + 1) * 128, qc * CH : (qc + 1) * CH],
                    )
            cs_sb = csn.tile([RD, N], BF16, tag="cs")
            nc.sync.dma_start(cs_sb[:], cs)
            sn_sb = csn.tile([RD, N], BF16, tag="sn")
            nc.sync.dma_start(sn_sb[:], sn)
            qkv_chunk(yt_sb, wkv_sb, 0, 0, N, N // 128, rope=False, CHc=M)
            for qc in range(N // CH):
                q0 = qc * CH
                qkv_chunk(xts[qc], wqkv_sb, NHL, q0, q0, q0 // 128, rope=True, CHc=CH)

        # ---------- phase C: attention + per-head A2A + projection ----------
        nc.scalar.activation(bias_sb[:, N // 128 : KT], ywc[:], AF.Ln)
        bpr_sb = pers.tile([1, C], F32, tag="bpr")
        nc.sync.dma_start(bpr_sb[:], bpr)
        bb_sb = pers.tile([128, C], F32, tag="bb")
        nc.gpsimd.partition_broadcast(bb_sb[:], bpr_sb[:])

        a2a_ins = [
            dram.tile([2 * NQC, 128, 512], BF16, name=f"a2ai{t}") for t in range(NHL)
        ]
        a2a_outs = [
            dram.tile([2 * NQC, 128, 512], BF16, name=f"a2ao{t}") for t in range(NHL)
        ]

        def a2a_head(t):
            nc.gpsimd.collective_compute(
                "AllToAll",
                ALU.bypass,
                replica_groups=[[0, 1, 2, 3, 4, 5, 6, 7]],
                ins=[a2a_ins[t].opt()],
                outs=[a2a_outs[t].opt()],
            )

        wpre = ctx.enter_context(tc.tile_pool(name="wpre", bufs=3))
        pjp = ctx.enter_context(tc.tile_pool(name="pjp", bufs=2))
        accp = ctx.enter_context(tc.tile_pool(name="accp", bufs=1))

        def prefetch_w(t, eng=None):
            # shared unmasked Wproj: row block (t*4+p) = Wproj rows of
            # head (3p+t); per head t keep [128, 4, C]
            wp = wpre.tile([128, 4, C], BF16, tag="wpre", name=f"wpre{t}")
            for p in range(4):
                r0 = (t * 4 + p) * 128
                (eng or nc.sync).dma_start(wp[:, p, :], wproj[r0 : r0 + 128, :])
            return wp

        with ExitStack() as pc:
            expp = pc.enter_context(tc.tile_pool(name="exp", bufs=6))
            exsp = pc.enter_context(tc.tile_pool(name="exs", bufs=4))
            brp2 = pc.enter_context(tc.tile_pool(name="bcast2", bufs=2))
            smallc = pc.enter_context(tc.tile_pool(name="smallc", bufs=2))
            avsp = pc.enter_context(tc.tile_pool(name="avs", bufs=2))
            psSc = pc.enter_context(tc.tile_pool(name="psSc", bufs=1, space="PSUM"))
            psAv = pc.enter_context(tc.tile_pool(name="psAv", bufs=1, space="PSUM"))
            psDen = pc.enter_context(tc.tile_pool(name="psDen", bufs=1, space="PSUM"))

            pend = {"cb": None}

            def flush_pend():
                cb = pend["cb"]
                if cb is not None:
                    pend["cb"] = None
                    cb()

            def attention_head(t, after_pair0=None):
                # two q-chunks (A/B) in lockstep: PE alternates between the
                # chunks so scalar-exp latency hides under matmuls, and each
                # kn/v stationary tile is reused for two matmuls
                for qp in range(2):
                    qA, qB = 2 * qp, 2 * qp + 1
                    avA = psAv.tile([128, 512], F32, tag="avA", name=f"avA{t}{qp}")
                    avB = psAv.tile([128, 512], F32, tag="avB", name=f"avB{t}{qp}")
                    den = psDen.tile([1, 1024], F32, tag="den", name=f"den{t}{qp}")
                    for kp in range(KT // 2):
                        scA = psSc.tile([128, 1024], F32, tag="scA")
                        scB = psSc.tile([128, 1024], F32, tag="scB")
                        for kh in range(2):
                            kt = 2 * kp + kh
                            for qc, sct in ((qA, scA), (qB, scB)):
                                nc.tensor.matmul(
                                    sct[:, kh * 512 : (kh + 1) * 512],
                                    kn[t][:, kt * 128 : (kt + 1) * 128],
                                    qn[t][:, qc * 512 : (qc + 1) * 512],
                                    start=True,
                                    stop=True,
                                )
                        if kp == 0:
                            # previous pair's deferred den finals + epilogue:
                            # emitted after this pair's first score matmuls so
                            # the PE never stalls on the vector-side quad sums
                            # (the new pair's first den write is at kp==1, so
                            # buffer-reuse ordering stays correct)
                            flush_pend()
                        if kp == 2 and qp == 0 and after_pair0 is not None:
                            after_pair0()
                        exA = expp.tile([128, 1024], BF16, tag="ex", name="exA")
                        exB = expp.tile([128, 1024], BF16, tag="ex", name="exB")
                        for ext, sct in ((exA, scA), (exB, scB)):
                            if kp < 8:
                                nc.scalar.activation(
                                    ext[:], sct[:], AF.Exp, bias=bias_sb[:, 0:1]
                                )
                            else:
                                for kh in range(2):
                                    kt = 2 * kp + kh
                                    nc.scalar.activation(
                                        ext[:, kh * 512 : (kh + 1) * 512],
                                        sct[:, kh * 512 : (kh + 1) * 512],
                                        AF.Exp,
                                        bias=bias_sb[:, kt : kt + 1],
                                    )
                        for kh in range(2):
                            kt = 2 * kp + kh
                            vsl = v_sb[
                                :,
                                kt * NHL * HD + t * HD : kt * NHL * HD + (t + 1) * HD,
                            ]
                            for avt, ext in ((avA, exA), (avB, exB)):
                                nc.tensor.matmul(
                                    avt[:],
                                    vsl,
                                    ext[:, kh * 512 : (kh + 1) * 512],
                                    start=(kt == 0),
                                    stop=(kt == KT - 1),
                                )
                        if kp == KT // 2 - 1:
                            # evacuate av PSUM first thing so the next pair's
                            # matmuls only wait on these copies
                            avsA = avsp.tile([128, 512], F32, tag="avsA")
                            nc.vector.tensor_copy(avsA[:], avA[:])
                            avsB = avsp.tile([128, 512], F32, tag="avsB")
                            nc.vector.tensor_copy(avsB[:], avB[:])
                        # den: pair then quad sums on vector, one accumulating
                        # den matmul per chunk per quad (PSUM-accumulated)
                        exsA = exsp.tile([128, 512], BF16, tag="exsA")
                        nc.vector.tensor_add(exsA[:], exA[:, 0:512], exA[:, 512:1024])
                        exsB = exsp.tile([128, 512], BF16, tag="exsB")
                        nc.vector.tensor_add(exsB[:], exB[:, 0:512], exB[:, 512:1024])
                        if kp % 2 == 0:
                            prevA, prevB = exsA, exsB
                        else:
                            exqA = exsp.tile([128, 512], BF16, tag="exqA", bufs=3)
                            nc.vector.tensor_add(exqA[:], prevA[:], exsA[:])
                            exqB = exsp.tile([128, 512], BF16, tag="exqB", bufs=3)
                            nc.vector.tensor_add(exqB[:], prevB[:], exsB[:])
                            if kp < KT // 2 - 1:
                                nc.tensor.matmul(
                                    den[:, 0:512], onesb_sb[:], exqA[:],
                                    start=(kp == 1), stop=False,
                                )
                                nc.tensor.matmul(
                                    den[:, 512:1024], onesb_sb[:], exqB[:],
                                    start=(kp == 1), stop=False,
                                )

                    def mk_ep(den=den, exqA=exqA, exqB=exqB, avsA=avsA,
                              avsB=avsB, t=t, qA=qA, qB=qB):
                        def ep():
                            nc.tensor.matmul(
                                den[:, 0:512], onesb_sb[:], exqA[:],
                                start=False, stop=True,
                            )
                            nc.tensor.matmul(
                                den[:, 512:1024], onesb_sb[:], exqB[:],
                                start=False, stop=True,
                            )
                            # 1/den via DVE Newton-Raphson approx (~2 ULP):
                            # keeps the scalar engine pure-Exp (an ACT
                            # table-set switch costs ~2.7us and exp/recip
                            # cannot share a table set)
                            rsc = smallc.tile([1, 1024], F32, tag="rsc")
                            invd = smallc.tile([1, 1024], F32, tag="invd")
                            nc.vector.reciprocal_approx_accurate(
                                invd[:], den[:], rsc[:]
                            )
                            bden = brp2.tile([128, 1024], F32, tag="bden")
                            nc.gpsimd.partition_broadcast(bden[:], invd[:])
                            # o{0,1} = av * mk{0,1} * bden, per-core mk in
                            # {0, 1}: exact-zero blocks go to the other
                            # batch's cores, letting the receiver fold batch
                            # halves with a plain add
                            for qc, avt, b0 in ((qA, avsA, 0), (qB, avsB, 512)):
                                o0 = outp.tile([128, 512], BF16, tag="o0")
                                nc.vector.scalar_tensor_tensor(
                                    o0[:], avt[:], mk_sb[:, 0:1],
                                    bden[:, b0 : b0 + 512],
                                    op0=ALU.mult, op1=ALU.mult,
                                )
                                o1 = outp.tile([128, 512], BF16, tag="o1")
                                nc.vector.scalar_tensor_tensor(
                                    o1[:], avt[:], mk_sb[:, 1:2],
                                    bden[:, b0 : b0 + 512],
                                    op0=ALU.mult, op1=ALU.mult,
                                )
                                nc.sync.dma_start(a2a_ins[t][qc], o0[:])
                                nc.sync.dma_start(a2a_ins[t][NQC + qc], o1[:])
                        return ep

                    pend["cb"] = mk_ep()

            def pj_fold(t, eng=None):
                # A2A result -> SBUF + fold batch halves (one half is zero);
                # emitted as early as the A2A completion allows so the tail
                # has only matmuls left
                pj_t = pjp.tile([128, 2 * NQC, 512], BF16, tag="pj", name=f"pj{t}")
                (eng or nc.sync).dma_start(
                    pj_t[:], a2a_outs[t].rearrange("i p q -> p i q")
                )
                pjs = pjp.tile([128, 4, 512], BF16, tag="pjs", name=f"pjs{t}")
                for p in range(4):
                    nc.vector.tensor_add(
                        pjs[:, p, :], pj_t[:, p, :], pj_t[:, 4 + p, :]
                    )
                return pjs

            acc = [
                accp.tile([128, 512], F32, tag=f"acc{i}", name=f"acc{i}")
                for i in range(12)
            ]

            def proj_partial(t, wp):
                # proj PSUM comes from the attention av rings (same shape/tag)
                # so there is no pool-scope transition barrier before the tail
                pjs = pjf[t]
                seq = 0
                for tcc in range(4):
                    for fc in range(3):
                        pp = psAv.tile(
                            [128, 512], F32,
                            tag=("avA" if seq % 2 == 0 else "avB"),
                            name=f"pp{t}_{fc}_{tcc}",
                        )
                        seq += 1
                        for p in range(4):
                            nc.tensor.matmul(
                                pp[:],
                                pjs[:, p, tcc * 128 : (tcc + 1) * 128],
                                wp[:, p, fc * 512 : (fc + 1) * 512],
                                start=(p == 0),
                                stop=(p == 3),
                            )
                        a = acc[fc * 4 + tcc]
                        if t == 0:
                            nc.vector.tensor_tensor(
                                a[:], pp[:],
                                bb_sb[:, fc * 512 : (fc + 1) * 512],
                                ALU.add,
                            )
                        elif t == 1:
                            nc.vector.tensor_add(a[:], a[:], pp[:])
                        else:
                            ob = outp.tile([128, 512], BF16, tag="ob")
                            nc.vector.tensor_add(ob[:], a[:], pp[:])
                            oeng = nc.sync if (fc + tcc) % 2 == 0 else nc.scalar
                            oeng.dma_start(
                                out[
                                    tcc * 128 : (tcc + 1) * 128,
                                    fc * 512 : (fc + 1) * 512,
                                ],
                                ob[:],
                            )

            wp0 = prefetch_w(0)
            attention_head(0)
            wp1 = prefetch_w(1)
            attention_head(1, after_pair0=lambda: a2a_head(0))
            # wp2/pj0 transfers go via the scalar engine's DMA queue so they
            # don't contend with o/a2a traffic on the sync queue at the
            # head1->head2 boundary
            wp2 = prefetch_w(2, eng=nc.scalar)
            pjf = {}

            def mid2():
                a2a_head(1)
                # a2a0 completed during head 1: fold it here so the tail's
                # first proj matmuls have their data the moment attention ends
                pjf[0] = pj_fold(0, eng=nc.scalar)

            attention_head(2, after_pair0=mid2)
            flush_pend()
            pjf[1] = pj_fold(1)
            a2a_head(2)
            proj_partial(0, wp0)
            proj_partial(1, wp1)
            # fold for head 2 emitted only now: its vector adds wait on the
            # last A2A and must not block proj0/1's accumulation adds
            pjf[2] = pj_fold(2)
            proj_partial(2, wp2)
    nc.compile()
    return nc


_NC_CACHE = {}


def _get_nc():
    if "nc" not in _NC_CACHE:
        _NC_CACHE["nc"] = build_nc()
    return _NC_CACHE["nc"]


def make_in_maps(x, y, pos, y_token_weights, Wqkv, Wkv, q_norm_w, k_norm_w, Wproj, bproj):
    f = np.float32
    c32 = pos[:, :, 0].T
    s32 = pos[:, :, 1].T
    csT = np.ascontiguousarray(
        np.concatenate([c32, c32], 0).astype(ml_dtypes.bfloat16))   # [64, N]
    snT = np.ascontiguousarray(
        np.concatenate([-s32, s32], 0).astype(ml_dtypes.bfloat16))  # [64, N]
    wqs = (np.asarray(q_norm_w, dtype=f) * np.float32(HD) ** -0.5).reshape(1, HD)
    wkk = np.asarray(k_norm_w, dtype=f).reshape(1, HD)
    Wp = np.asarray(Wproj, dtype=f)
    # shared unmasked proj weights: row block (t*4+p) = rows of head (3p+t)
    wproj_tg = np.concatenate(
        [Wp[(3 * p + t) * 128 : (3 * p + t + 1) * 128, :]
         for t in range(NHL) for p in range(4)],
        axis=0,
    ).astype(ml_dtypes.bfloat16)
    wproj_tg = np.ascontiguousarray(wproj_tg)
    in_maps = []
    for c in range(8):
        b, g = c // 4, c % 4
        heads = [3 * g + i for i in range(NHL)]
        qcols = [Wqkv[:, h * HD : (h + 1) * HD] for h in heads]
        kcols = [Wqkv[:, C + h * HD : C + (h + 1) * HD] for h in heads]
        vcols = [Wqkv[:, 2 * C + h * HD : 2 * C + (h + 1) * HD] for h in heads]
        wqkv_c = np.ascontiguousarray(
            np.concatenate(qcols + kcols + vcols, axis=1), dtype=f
        )
        kcols2 = [Wkv[:, h * HD : (h + 1) * HD] for h in heads]
        vcols2 = [Wkv[:, C + h * HD : C + (h + 1) * HD] for h in heads]
        wkv_c = np.ascontiguousarray(np.concatenate(kcols2 + vcols2, axis=1), dtype=f)
        mk_c = np.zeros((128, 2), dtype=f)
        mk_c[:, b] = 1.0
        in_maps.append(
            {
                "xT": np.ascontiguousarray(np.asarray(x)[b].T.astype(ml_dtypes.bfloat16)),
                "yT": np.ascontiguousarray(np.asarray(y)[b].T.astype(ml_dtypes.bfloat16)),
                "wqkv": wqkv_c.astype(ml_dtypes.bfloat16),
                "wkv": wkv_c.astype(ml_dtypes.bfloat16),
                "wproj": wproj_tg,
                "wq": np.ascontiguousarray(wqs),
                "wk": np.ascontiguousarray(wkk),
                "cs": csT,
                "sn": snT,
                "ywT": np.ascontiguousarray(
                    np.asarray(y_token_weights)[b].reshape(M // 128, 128).T, dtype=f
                ),
                "bpr": np.asarray(bproj, dtype=f).reshape(1, C),
                "mk": mk_c,
            }
        )
    return in_maps


def kernel(x, y, pos, y_token_weights, Wqkv, Wkv, q_norm_w, k_norm_w, Wproj, bproj,
           _trace=False):
    x = np.asarray(x, dtype=np.float32)
    y = np.asarray(y, dtype=np.float32)
    pos = np.asarray(pos, dtype=np.float32)
    y_token_weights = np.asarray(y_token_weights, dtype=np.float32)
    nc = _get_nc()
    in_maps = make_in_maps(
        x, y, pos, y_token_weights,
        np.asarray(Wqkv), np.asarray(Wkv), np.asarray(q_norm_w),
        np.asarray(k_norm_w), np.asarray(Wproj), np.asarray(bproj),
    )
    res = run_bass_kernel_spmd(nc, in_maps, core_ids=list(range(8)), trace=_trace)
    outp = np.zeros((B, N, C), dtype=np.float32)
    for c in range(8):
        b, g = c // 4, c % 4
        outp[b, g * 512 : (g + 1) * 512, :] = np.asarray(
            res.results[c]["out"]
        ).astype(np.float32)
    if _trace:
        return outp, res
    return outp
